# revision 10
# baseline (speedup 1.0000x reference)
"""Trainium2 Bass kernel: Deepseek-style decoder layer (dense transformer),
tensor-parallel over 8 NeuronCores.

Per core: 2 Q heads + their GQA KV head; attention computed in transposed
(scoresT) layout so softmax-denominators come from ones-matmuls and no probs
transpose is needed. Attention head outputs are AllToAll'd so each core gets
all heads for its 256-token sequence shard; o_proj/residual/norm2 run
seq-sharded; h2 shards are AllGathered for the tensor-parallel gate/up
matmuls (1024 FF cols/core); gated activations are AllToAll'd back to
seq-shards for the full down_proj. RMSNorm weights fold into following
projections on host; rstd row-scales fold into RoPE multipliers/epilogues.
All matmuls fp16 (full PE rate), fp32 PSUM accumulation + fp32 residuals.
"""
import sys
import os
import numpy as np

for _p in ("/opt/trn_rl_repo", "/root/.axon_site/_ro/trn_rl_repo"):
    if os.path.isdir(_p) and _p not in sys.path:
        sys.path.append(_p)

B, S, D = 1, 2048, 2048
H, KVH, HD = 16, 4, 128
FF = 8192
EPS = 1e-6
THETA = 10000.0
NC = 8
P = 128
SBLK = 512
NSB = S // SBLK          # 4 s-blocks
DCH = D // P             # 16 d-chunks
SHARD = S // NC          # 256 tokens per core
FFSH = FF // NC          # 1024
NKC = S // P             # 16 sk-chunks
EXP_SHIFT = -8.0         # exp(x+shift): cancels in softmax ratio, keeps fp16
                         # probs far from overflow without a max pass

_CACHE = {}


def _install_drain_patch(tile_mod, mybir):
    """Stock tail-drain puts one wait per outstanding proc on a single sync-
    queue CTRL op, which supports only ONE wait -> chain single-wait drains."""
    from concourse.vector_clock import ScopedClock

    def _split(self, tick_clock, wait_clock):
        nc = self.nc
        drain_inst = nc.sync.drain()
        wait_clock.add_sem_waits(
            drain_inst.ins, ScopedClock({None: tick_clock.global_clock}))
        si = drain_inst.ins.sync_info
        if si is not None and len(si.on_wait) > 1:
            waits, upd = list(si.on_wait), list(si.on_update)
            drain_inst.ins.sync_info = mybir.SyncInfo(
                on_wait=waits[:1], on_update=[])
            rest = waits[1:]
            while rest:
                chunk, rest = rest[:1], rest[1:]
                extra = nc.sync.drain()
                extra.ins.sync_info = mybir.SyncInfo(
                    on_wait=chunk, on_update=([] if rest else upd))
        nc.all_engine_barrier()
        assert self.sems is not None
        popped = nc._tile_sem_poison_stack.pop()
        assert popped is self._sem_poison
        nc.clear_and_free_semaphores(list(self.sems.allocated().values()))
        nc.all_engine_barrier()

    tile_mod.TileContext._drain_and_barrier = _split


def _split_waits(nc, mybir):
    """HW allows one sync-wait per instruction on these queues; Tile can emit
    several at dependency joins. Insert same-engine no-ops, each carrying one
    excess wait, immediately before the offending instruction."""
    eng_map = {
        mybir.EngineType.DVE: nc.vector,
        mybir.EngineType.Activation: nc.scalar,
        mybir.EngineType.PE: nc.tensor,
        mybir.EngineType.Pool: nc.gpsimd,
        mybir.EngineType.SP: nc.sync,
    }
    for bb in nc.main_func.blocks:
        todo = []
        for inst in bb.instructions:
            si = getattr(inst, "sync_info", None)
            if si is not None and len(si.on_wait) > 1:
                todo.append(inst)
        if not todo:
            continue
        inserts = {}
        created = []
        for inst in todo:
            si = inst.sync_info
            waits = list(si.on_wait)
            nops = []
            for w in waits[:-1]:
                nop = eng_map[inst.engine].nop().ins
                nop.sync_info = mybir.SyncInfo(on_wait=[w], on_update=[])
                nops.append(nop)
                created.append(nop)
            inst.sync_info = mybir.SyncInfo(
                on_wait=[waits[-1]], on_update=list(si.on_update))
            inserts[id(inst)] = nops
        created_ids = {id(n) for n in created}
        # nops were appended to the current block; rebuild every block,
        # dropping stray nops and splicing them before their target.
        for bb2 in nc.main_func.blocks:
            out = []
            for inst in bb2.instructions:
                if id(inst) in created_ids:
                    continue
                out.extend(inserts.get(id(inst), ()))
                out.append(inst)
            bb2.instructions[:] = out


def _classify_mask(maskT):
    """Tile class per (ik sk-chunk, jq sq-block) of maskT [sk, sq]."""
    cls = {}
    sk = np.arange(S)[:, None]
    sq = np.arange(S)[None, :]
    causal = np.where(sq >= sk, 0.0, -1e9).astype(np.float32)
    for ik in range(NKC):
        for jq in range(NSB):
            t = maskT[ik * P:(ik + 1) * P, jq * SBLK:(jq + 1) * SBLK]
            if np.all(t <= -1e8):
                cls[(ik, jq)] = "SKIP"
            elif np.all(t == 0.0):
                cls[(ik, jq)] = "FREE"
            elif np.array_equal(
                    t, causal[ik * P:(ik + 1) * P, jq * SBLK:(jq + 1) * SBLK]):
                cls[(ik, jq)] = "DIAG"
            else:
                cls[(ik, jq)] = "MIX"
    return cls


def _build(cls_key, cls):
    import concourse.bass as bass
    import concourse.mybir as mybir
    import concourse.tile as tile
    from concourse import masks

    _install_drain_patch(tile, mybir)
    f16, f32 = mybir.dt.float16, mybir.dt.float32
    nc = bass.Bass("TRN2", target_bir_lowering=False, debug=False,
                   num_devices=NC)

    di = lambda n, s: nc.dram_tensor(n, s, f16, kind="ExternalInput").ap()
    df = lambda n, s: nc.dram_tensor(n, s, f32, kind="ExternalInput").ap()

    xt = di("xt", [NSB, P, DCH, SBLK])      # packed xT fp16 per s-block
    xc = df("xc", [D, SHARD])               # xT fp32, this core's shard cols
    cost = df("cost", [P, S])               # rope cos  [hd, s]
    s2t = df("s2t", [P, S])                 # rope sign-folded sin [hd, s]
    wq = di("wq", [P, DCH, 2 * P])
    wk = di("wk", [P, DCH, P])
    wv = di("wv", [P, DCH, P])
    wo = di("wo", [DCH, P, DCH, P])         # [dtile][p][echunk][m]
    wg = di("wg", [2, P, DCH, SBLK])        # [ffhalf][p][dchunk][ff]
    wu = di("wu", [2, P, DCH, SBLK])
    wd = di("wd", [DCH, P, FF // P, P])     # [dtile][p][ffchunk][m]
    maskt = df("maskt", [S, S])
    outt = nc.dram_tensor("outt", [D, SHARD], f32, kind="ExternalOutput").ap()

    with tile.TileContext(nc) as tc:
        with (
            tc.tile_pool(name="pers", bufs=1) as pers,
            tc.tile_pool(name="dram", bufs=1, space="DRAM") as dram,
        ):
            ones = pers.tile([P, P], f16, name="ones")
            nc.vector.memset(ones[:], 1.0)
            ident = pers.tile([P, P], f16, name="ident")
            masks.make_identity(nc, ident[:])
            permT = pers.tile([P, P], f16, name="permT")  # rot-half permute
            nc.gpsimd.memset(permT[:], 0.0)
            for base in (-64, 64):
                nc.gpsimd.affine_select(
                    out=permT[:], in_=permT[:],
                    compare_op=mybir.AluOpType.not_equal,
                    fill=1.0, base=base, pattern=[[-1, P]],
                    channel_multiplier=1)

            shift = pers.tile([P, 1], f32, name="shift")
            nc.vector.memset(shift[:], EXP_SHIFT)
            x1t = pers.tile([P, DCH, SHARD], f32, name="x1t")

            a2a1_in = dram.tile([NC, 2 * P, SHARD], f16)
            a2a1_out = dram.tile([NC, 2 * P, SHARD], f16)
            ag2_in = dram.tile([D, SHARD], f16)
            ag2_out = dram.tile([NC * D, SHARD], f16, addr_space="Shared")
            a2a3_in = dram.tile([NC, FFSH, SHARD], f16)
            a2a3_out = dram.tile([NC, FFSH, SHARD], f16)

            # ======== A: norm1 stats + QKV + RoPE + v transpose ========
            _pAB_cm = tc.tile_pool(name="pAB", bufs=1)
            pAB = _pAB_cm.__enter__()
            qh = [pAB.tile([P, S], f16, name=f"qh{i}") for i in range(2)]
            kh = pAB.tile([P, S], f16, name="kh")
            vnat = pAB.tile([P, NKC, P], f16, name="vnat")
            attn_st = pAB.tile([P, 2, S], f16, name="attn_st")
            with (
                tc.tile_pool(name="pA", bufs=2) as pA,
                tc.tile_pool(name="pAx", bufs=2) as pAx,
                tc.tile_pool(name="pAw", bufs=1) as pAw,
                tc.tile_pool(name="psA", bufs=2, space="PSUM") as psA,
                tc.tile_pool(name="psAv", bufs=2, space="PSUM") as psAv,
            ):
                wq_sb = pAw.tile([P, DCH, 2 * P], f16, name="wq_sb")
                nc.sync.dma_start(wq_sb[:], wq[:])
                wk_sb = pAw.tile([P, DCH, P], f16, name="wk_sb")
                nc.sync.dma_start(wk_sb[:], wk[:])
                wv_sb = pAw.tile([P, DCH, P], f16, name="wv_sb")
                nc.sync.dma_start(wv_sb[:], wv[:])

                for j in range(NSB):
                    sl = slice(j * SBLK, (j + 1) * SBLK)
                    xt_sb = pAx.tile([P, DCH, SBLK], f16, name="xt_sb")
                    nc.sync.dma_start(xt_sb[:], xt[j])
                    var_ps = psAv.tile([P, SBLK], f32, name="var")
                    for i in range(DCH):
                        xsq = pA.tile([P, SBLK], f16, name="xsq")
                        nc.vector.tensor_mul(xsq[:], xt_sb[:, i, :],
                                             xt_sb[:, i, :])
                        nc.tensor.matmul(var_ps[:], ones[:], xsq[:],
                                         start=(i == 0), stop=(i == DCH - 1))
                    t1 = pA.tile([P, SBLK], f32, name="t1")
                    nc.vector.tensor_scalar(
                        t1[:], var_ps[:], 1.0 / D, EPS,
                        mybir.AluOpType.mult, mybir.AluOpType.add)
                    t2 = pA.tile([P, SBLK], f32, name="t2")
                    nc.vector.reciprocal(t2[:], t1[:])
                    rstd = pA.tile([P, SBLK], f32, name="rstd")
                    nc.scalar.sqrt(rstd[:], t2[:])
                    cosj = pA.tile([P, SBLK], f32, name="cosj")
                    nc.sync.dma_start(cosj[:], cost[:, sl])
                    s2j = pA.tile([P, SBLK], f32, name="s2j")
                    nc.sync.dma_start(s2j[:], s2t[:, sl])
                    cr = pA.tile([P, SBLK], f32, name="cr")
                    nc.vector.tensor_mul(cr[:], cosj[:], rstd[:])
                    sr = pA.tile([P, SBLK], f32, name="sr")
                    nc.vector.tensor_mul(sr[:], s2j[:], rstd[:])

                    for (wsb, col0, dst) in (
                        (wq_sb, 0, qh[0]), (wq_sb, P, qh[1]), (wk_sb, 0, kh)
                    ):
                        ps = psA.tile([P, SBLK], f32, name="mm")
                        for i in range(DCH):
                            nc.tensor.matmul(
                                ps[:], wsb[:, i, col0:col0 + P],
                                xt_sb[:, i, :],
                                start=(i == 0), stop=(i == DCH - 1))
                        z16 = pA.tile([P, SBLK], f16, name="z16")
                        nc.vector.tensor_copy(z16[:], ps[:])
                        rps = psA.tile([P, SBLK], f32, name="rot")
                        nc.tensor.matmul(rps[:], permT[:], z16[:],
                                         start=True, stop=True)
                        av = pA.tile([P, SBLK], f32, name="av")
                        nc.vector.tensor_mul(av[:], ps[:], cr[:])
                        bv = pA.tile([P, SBLK], f32, name="bv")
                        nc.vector.tensor_mul(bv[:], rps[:], sr[:])
                        nc.vector.tensor_add(dst[:, sl], av[:], bv[:])

                    ps = psA.tile([P, SBLK], f32, name="mm")
                    for i in range(DCH):
                        nc.tensor.matmul(ps[:], wv_sb[:, i, :], xt_sb[:, i, :],
                                         start=(i == 0), stop=(i == DCH - 1))
                    vs = pA.tile([P, SBLK], f16, name="vs")
                    nc.vector.tensor_mul(vs[:], ps[:], rstd[:])
                    for t in range(SBLK // P):
                        tps = psAv.tile([P, P], f16, name="vt")
                        nc.tensor.transpose(tps[:], vs[:, t * P:(t + 1) * P],
                                            ident[:])
                        nc.vector.tensor_copy(
                            vnat[:, j * (SBLK // P) + t, :], tps[:])

            # ======== B: attention in scoresT layout ========
            with (
                tc.tile_pool(name="pB", bufs=4) as pB,
                tc.tile_pool(name="psB", bufs=3, space="PSUM") as psB,
                tc.tile_pool(name="psBa", bufs=2, space="PSUM") as psBa,
            ):
                for h in range(2):
                    for jq in range(NSB):
                        slq = slice(jq * SBLK, (jq + 1) * SBLK)
                        live = [ik for ik in range(NKC)
                                if cls[(ik, jq)] != "SKIP"]
                        dn_ps = psBa.tile([P, SBLK], f32, name="dn")
                        at_ps = psBa.tile([P, SBLK], f32, name="at")
                        for n, ik in enumerate(live):
                            c = cls[(ik, jq)]
                            sc = psB.tile([P, SBLK], f32, name="sc")
                            nc.tensor.matmul(
                                sc[:], kh[:, ik * P:(ik + 1) * P],
                                qh[h][:, slq], start=True, stop=True)
                            pr = pB.tile([P, SBLK], f16, name="pr")
                            if c == "FREE":
                                nc.scalar.activation(
                                    pr[:], sc[:],
                                    mybir.ActivationFunctionType.Exp,
                                    bias=shift[:, :])
                            elif c == "DIAG":
                                cp = pB.tile([P, SBLK], f32, name="cp")
                                nc.vector.tensor_copy(cp[:], sc[:])
                                nc.gpsimd.affine_select(
                                    out=cp[:], in_=cp[:],
                                    compare_op=mybir.AluOpType.is_ge,
                                    fill=-1e5, base=jq * SBLK - ik * P,
                                    pattern=[[1, SBLK]], channel_multiplier=-1)
                                nc.scalar.activation(
                                    pr[:], cp[:],
                                    mybir.ActivationFunctionType.Exp,
                                    bias=shift[:, :])
                            else:  # MIX
                                mk = pB.tile([P, SBLK], f32, name="mk")
                                nc.sync.dma_start(
                                    mk[:], maskt[ik * P:(ik + 1) * P, slq])
                                cp = pB.tile([P, SBLK], f32, name="cp")
                                nc.vector.tensor_add(cp[:], sc[:], mk[:])
                                nc.scalar.activation(
                                    pr[:], cp[:],
                                    mybir.ActivationFunctionType.Exp,
                                    bias=shift[:, :])
                            nc.tensor.matmul(dn_ps[:], ones[:], pr[:],
                                             start=(n == 0),
                                             stop=(n == len(live) - 1))
                            nc.tensor.matmul(at_ps[:], vnat[:, ik, :], pr[:],
                                             start=(n == 0),
                                             stop=(n == len(live) - 1))
                        rc = pB.tile([P, SBLK], f32, name="rc")
                        nc.vector.reciprocal(rc[:], dn_ps[:])
                        nc.vector.tensor_mul(attn_st[:, h, slq], at_ps[:],
                                             rc[:])

            for h in range(2):
                nc.sync.dma_start(
                    a2a1_in[:].rearrange("c (h p) s -> p h c s", p=P)[:, h],
                    attn_st[:, h].rearrange("p (c s) -> p c s", c=NC))
            nc.gpsimd.collective_compute(
                "AllToAll", mybir.AluOpType.bypass,
                replica_groups=[list(range(NC))],
                ins=[a2a1_in[:].opt()], outs=[a2a1_out[:].opt()])
            _pAB_cm.__exit__(None, None, None)

            # ======== C: o_proj (seq-shard) + residual + norm2 ========
            with (
                tc.tile_pool(name="pC", bufs=3) as pC,
                tc.tile_pool(name="pCr", bufs=1) as pCr,
                tc.tile_pool(name="psC", bufs=2, space="PSUM") as psC,
                tc.tile_pool(name="psCv", bufs=1, space="PSUM") as psCv,
            ):
                attn_rb = pCr.tile([P, DCH, SHARD], f16, name="attn_rb")
                nc.sync.dma_start(
                    attn_rb[:],
                    a2a1_out[:].rearrange("c (h p) s -> p (c h) s", p=P))
                xc_sb = pCr.tile([P, DCH, SHARD], f32, name="xc_sb")
                nc.sync.dma_start(
                    xc_sb[:], xc[:].rearrange("(n p) s -> p n s", p=P))
                var2 = psCv.tile([P, SHARD], f32, name="var2")
                for i in range(DCH):
                    wo_sb = pC.tile([P, DCH, P], f16, name="wo_sb")
                    nc.sync.dma_start(wo_sb[:], wo[i])
                    ps = psC.tile([P, SHARD], f32, name="wops")
                    for e in range(DCH):
                        nc.tensor.matmul(ps[:], wo_sb[:, e, :],
                                         attn_rb[:, e, :],
                                         start=(e == 0), stop=(e == DCH - 1))
                    nc.vector.tensor_add(x1t[:, i, :], ps[:], xc_sb[:, i, :])
                    sq2 = pC.tile([P, SHARD], f16, name="sq2")
                    nc.vector.tensor_mul(sq2[:], x1t[:, i, :], x1t[:, i, :])
                    nc.tensor.matmul(var2[:], ones[:], sq2[:],
                                     start=(i == 0), stop=(i == DCH - 1))
                u1 = pCr.tile([P, SHARD], f32, name="u1")
                nc.vector.tensor_scalar(
                    u1[:], var2[:], 1.0 / D, EPS,
                    mybir.AluOpType.mult, mybir.AluOpType.add)
                u2 = pCr.tile([P, SHARD], f32, name="u2")
                nc.vector.reciprocal(u2[:], u1[:])
                rstd2 = pCr.tile([P, SHARD], f32, name="rstd2")
                nc.scalar.sqrt(rstd2[:], u2[:])
                h2st = pCr.tile([P, DCH, SHARD], f16, name="h2st")
                for i in range(DCH):
                    nc.vector.tensor_mul(h2st[:, i, :], x1t[:, i, :],
                                         rstd2[:])
                nc.sync.dma_start(
                    ag2_in[:].rearrange("(n p) s -> p n s", p=P), h2st[:])
            nc.gpsimd.collective_compute(
                "AllGather", mybir.AluOpType.bypass,
                replica_groups=[list(range(NC))],
                ins=[ag2_in[:].opt()], outs=[ag2_out[:].opt()])

            # ======== D: gate/up (ff-shard) + silu*up ========
            with (
                tc.tile_pool(name="pD", bufs=2) as pD,
                tc.tile_pool(name="pDw", bufs=1) as pDw,
                tc.tile_pool(name="pDg", bufs=1) as pDg,
                tc.tile_pool(name="psD", bufs=4, space="PSUM") as psD,
            ):
                gstage = pDg.tile([P, FFSH // P, S], f16, name="gstage")
                for half in range(2):
                    wg_sb = pDw.tile([P, DCH, SBLK], f16, name="wg_sb")
                    nc.sync.dma_start(wg_sb[:], wg[half])
                    wu_sb = pDw.tile([P, DCH, SBLK], f16, name="wu_sb")
                    nc.sync.dma_start(wu_sb[:], wu[half])
                    for j in range(NSB):
                        h2rb = pD.tile([P, DCH, 2, SHARD], f16, name="h2rb")
                        for cc in range(2):
                            nc.sync.dma_start(
                                h2rb[:, :, cc, :],
                                ag2_out[:].rearrange(
                                    "(c n p) s -> p n c s", c=NC, p=P
                                )[:, :, 2 * j + cc, :])
                        for ft in range(SBLK // P):
                            fsl = slice(ft * P, (ft + 1) * P)
                            psg = psD.tile([P, SBLK], f32, name="psg")
                            for i in range(DCH):
                                nc.tensor.matmul(
                                    psg[:], wg_sb[:, i, fsl], h2rb[:, i],
                                    start=(i == 0), stop=(i == DCH - 1))
                            psu = psD.tile([P, SBLK], f32, name="psu")
                            for i in range(DCH):
                                nc.tensor.matmul(
                                    psu[:], wu_sb[:, i, fsl], h2rb[:, i],
                                    start=(i == 0), stop=(i == DCH - 1))
                            slv = pD.tile([P, SBLK], f32, name="slv")
                            nc.scalar.activation(
                                slv[:], psg[:],
                                mybir.ActivationFunctionType.Silu)
                            nc.vector.tensor_mul(
                                gstage[:, half * 4 + ft,
                                       j * SBLK:(j + 1) * SBLK],
                                slv[:], psu[:])
                for n in range(FFSH // P):
                    nc.sync.dma_start(
                        a2a3_in[:].rearrange("c (n p) s -> p n c s", p=P)[:, n],
                        gstage[:, n].rearrange("p (c s) -> p c s", c=NC))
            nc.gpsimd.collective_compute(
                "AllToAll", mybir.AluOpType.bypass,
                replica_groups=[list(range(NC))],
                ins=[a2a3_in[:].opt()], outs=[a2a3_out[:].opt()])

            # ======== E: down_proj (seq-shard) + final residual ========
            with (
                tc.tile_pool(name="pE", bufs=2) as pE,
                tc.tile_pool(name="pEg", bufs=1) as pEg,
                tc.tile_pool(name="psE", bufs=2, space="PSUM") as psE,
            ):
                grb = pEg.tile([P, FF // P, SHARD], f16, name="grb")
                nc.sync.dma_start(
                    grb[:],
                    a2a3_out[:].rearrange("c (n p) s -> p (c n) s", p=P))
                outst = pEg.tile([P, DCH, SHARD], f32, name="outst")
                for i in range(DCH):
                    wd_sb = pE.tile([P, FF // P, P], f16, name="wd_sb")
                    nc.sync.dma_start(wd_sb[:], wd[i])
                    ps = psE.tile([P, SHARD], f32, name="dps")
                    for f in range(FF // P):
                        nc.tensor.matmul(ps[:], wd_sb[:, f, :], grb[:, f, :],
                                         start=(f == 0),
                                         stop=(f == FF // P - 1))
                    nc.vector.tensor_add(outst[:, i, :], ps[:], x1t[:, i, :])
                nc.sync.dma_start(
                    outt[:].rearrange("(n p) s -> p n s", p=P), outst[:])
    import concourse.mybir as _mybir
    _split_waits(nc, _mybir)
    return nc


def _host_prep(inputs):
    x = np.ascontiguousarray(inputs["hidden_states"][0])          # [S, D]
    mask = np.ascontiguousarray(inputs["attention_mask"][0, 0])   # [sq, sk]
    maskT = np.ascontiguousarray(mask.T)                          # [sk, sq]
    ln1, ln2 = inputs["ln1_w"], inputs["ln2_w"]
    Wq, Wk, Wv, Wo = inputs["Wq"], inputs["Wk"], inputs["Wv"], inputs["Wo"]
    Wg, Wu, Wd = inputs["Wg"], inputs["Wu"], inputs["Wd"]

    xT = np.ascontiguousarray(x.T)                                # [D, S]
    xT16 = xT.astype(np.float16)
    # packed xt: [j sblk][p][n dchunk][m] = xT[n*128+p, j*512+m]
    xtp = np.ascontiguousarray(
        xT16.reshape(DCH, P, NSB, SBLK).transpose(2, 1, 0, 3))

    inv_freq = 1.0 / (THETA ** (np.arange(0, HD, 2, dtype=np.float32) / HD))
    t = np.arange(S, dtype=np.float32)
    freqs = np.outer(t, inv_freq)
    emb = np.concatenate([freqs, freqs], -1)                      # [S, HD]
    cosT = np.ascontiguousarray(np.cos(emb).T.astype(np.float32))  # [HD, S]
    sinT = np.sin(emb).T.astype(np.float32)
    s2T = sinT.copy()
    s2T[:64] = -s2T[:64]
    s2T = np.ascontiguousarray(s2T)

    scale = 1.0 / np.sqrt(HD)
    Wq_f = (ln1[:, None] * Wq * scale).astype(np.float16)   # [D, H*HD]
    Wk_f = (ln1[:, None] * Wk).astype(np.float16)
    Wv_f = (ln1[:, None] * Wv).astype(np.float16)
    Wg_f = (ln2[:, None] * Wg).astype(np.float16)
    Wu_f = (ln2[:, None] * Wu).astype(np.float16)
    Wo16 = Wo.astype(np.float16)                            # [H*HD, D]
    Wd16 = Wd.astype(np.float16)                            # [FF, D]

    # packed wo: [i dtile][p][e chunk][m] = Wo[e*128+p, i*128+m]
    wop = np.ascontiguousarray(
        Wo16.reshape(DCH, P, DCH, P).transpose(2, 1, 0, 3))
    # packed wd: [i][p][f chunk][m] = Wd[f*128+p, i*128+m]
    wdp = np.ascontiguousarray(
        Wd16.reshape(FF // P, P, DCH, P).transpose(2, 1, 0, 3))

    cls = _classify_mask(maskT)
    in_maps = []
    for c in range(NC):
        qsl = slice(2 * P * c, 2 * P * (c + 1))
        kvsl = slice(P * (c // 2), P * (c // 2) + P)
        ffsl = slice(FFSH * c, FFSH * (c + 1))
        ssl = slice(SHARD * c, SHARD * (c + 1))
        wq_c = Wq_f[:, qsl]    # [D, 256]
        wk_c = Wk_f[:, kvsl]   # [D, 128]
        wv_c = Wv_f[:, kvsl]
        wg_c = Wg_f[:, ffsl]   # [D, 1024]
        wu_c = Wu_f[:, ffsl]
        in_maps.append({
            "xt": xtp,
            "xc": np.ascontiguousarray(xT[:, ssl]),
            "cost": cosT,
            "s2t": s2T,
            # [p][n dchunk][cols]
            "wq": np.ascontiguousarray(
                wq_c.reshape(DCH, P, 2 * P).transpose(1, 0, 2)),
            "wk": np.ascontiguousarray(
                wk_c.reshape(DCH, P, P).transpose(1, 0, 2)),
            "wv": np.ascontiguousarray(
                wv_c.reshape(DCH, P, P).transpose(1, 0, 2)),
            "wo": wop,
            # [half][p][n dchunk][ff 512] = Wg_f[n*128+p, half*512+m]
            "wg": np.ascontiguousarray(
                wg_c.reshape(DCH, P, 2, SBLK).transpose(2, 1, 0, 3)),
            "wu": np.ascontiguousarray(
                wu_c.reshape(DCH, P, 2, SBLK).transpose(2, 1, 0, 3)),
            "wd": wdp,
            "maskt": maskT,
        })
    return in_maps, cls


def kernel(**inputs):
    from concourse import bass_utils

    in_maps, cls = _host_prep(inputs)
    cls_key = tuple(sorted(cls.items()))
    if cls_key not in _CACHE:
        _CACHE[cls_key] = _build(cls_key, cls)
    nc = _CACHE[cls_key]

    res = bass_utils.run_bass_kernel_spmd(
        nc, in_maps, core_ids=list(range(NC)))
    out = np.empty((S, D), dtype=np.float32)
    for c in range(NC):
        out[SHARD * c:SHARD * (c + 1), :] = res.results[c]["outt"].T
    return out[None]


# revision 22
# speedup vs baseline: 14003.4858x; 14003.4858x over previous
"""Trainium2 Bass kernel: Deepseek-style decoder layer (dense transformer),
tensor-parallel over 8 NeuronCores.

Per core: 2 Q heads + their GQA KV head; attention computed in transposed
(scoresT) layout so softmax-denominators come from ones-matmuls and no probs
transpose is needed. Attention head outputs are AllToAll'd so each core gets
all heads for its 256-token sequence shard; o_proj/residual/norm2 run
seq-sharded; h2 shards are AllGathered for the tensor-parallel gate/up
matmuls (1024 FF cols/core); gated activations are AllToAll'd back to
seq-shards for the full down_proj. RMSNorm weights fold into following
projections on host; rstd row-scales fold into RoPE multipliers/epilogues.
All matmuls fp16 (full PE rate), fp32 PSUM accumulation + fp32 residuals.
"""
import sys
import os
import numpy as np

for _p in ("/opt/trn_rl_repo", "/root/.axon_site/_ro/trn_rl_repo"):
    if os.path.isdir(_p) and _p not in sys.path:
        sys.path.append(_p)

B, S, D = 1, 2048, 2048
H, KVH, HD = 16, 4, 128
FF = 8192
EPS = 1e-6
THETA = 10000.0
NC = 8
P = 128
SBLK = 512
NSB = S // SBLK          # 4 s-blocks
DCH = D // P             # 16 d-chunks
SHARD = S // NC          # 256 tokens per core
FFSH = FF // NC          # 1024
NKC = S // P             # 16 sk-chunks
EXP_SHIFT = -8.0         # exp(x+shift): cancels in softmax ratio, keeps fp16
                         # probs far from overflow without a max pass

_CACHE = {}


def _install_drain_patch(tile_mod, mybir):
    """Stock tail-drain puts one wait per outstanding proc on a single sync-
    queue CTRL op, which supports only ONE wait -> chain single-wait drains."""
    from concourse.vector_clock import ScopedClock

    def _split(self, tick_clock, wait_clock):
        nc = self.nc
        drain_inst = nc.sync.drain()
        wait_clock.add_sem_waits(
            drain_inst.ins, ScopedClock({None: tick_clock.global_clock}))
        si = drain_inst.ins.sync_info
        if si is not None and len(si.on_wait) > 1:
            waits, upd = list(si.on_wait), list(si.on_update)
            drain_inst.ins.sync_info = mybir.SyncInfo(
                on_wait=waits[:1], on_update=[])
            rest = waits[1:]
            while rest:
                chunk, rest = rest[:1], rest[1:]
                extra = nc.sync.drain()
                extra.ins.sync_info = mybir.SyncInfo(
                    on_wait=chunk, on_update=([] if rest else upd))
        nc.all_engine_barrier()
        assert self.sems is not None
        popped = nc._tile_sem_poison_stack.pop()
        assert popped is self._sem_poison
        nc.clear_and_free_semaphores(list(self.sems.allocated().values()))
        nc.all_engine_barrier()

    tile_mod.TileContext._drain_and_barrier = _split


def _split_waits(nc, mybir):
    """HW allows one sync-wait per instruction on these queues; Tile can emit
    several at dependency joins. Insert same-engine no-ops, each carrying one
    excess wait, immediately before the offending instruction."""
    eng_map = {
        mybir.EngineType.DVE: nc.vector,
        mybir.EngineType.Activation: nc.scalar,
        mybir.EngineType.PE: nc.tensor,
        mybir.EngineType.Pool: nc.gpsimd,
        mybir.EngineType.SP: nc.sync,
    }
    for bb in nc.main_func.blocks:
        todo = []
        for inst in bb.instructions:
            si = getattr(inst, "sync_info", None)
            if si is not None and len(si.on_wait) > 1:
                todo.append(inst)
        if not todo:
            continue
        inserts = {}
        created = []
        for inst in todo:
            si = inst.sync_info
            waits = list(si.on_wait)
            nops = []
            for w in waits[:-1]:
                nop = eng_map[inst.engine].nop().ins
                nop.sync_info = mybir.SyncInfo(on_wait=[w], on_update=[])
                nops.append(nop)
                created.append(nop)
            inst.sync_info = mybir.SyncInfo(
                on_wait=[waits[-1]], on_update=list(si.on_update))
            inserts[id(inst)] = nops
        created_ids = {id(n) for n in created}
        # nops were appended to the current block; rebuild every block,
        # dropping stray nops and splicing them before their target.
        for bb2 in nc.main_func.blocks:
            out = []
            for inst in bb2.instructions:
                if id(inst) in created_ids:
                    continue
                out.extend(inserts.get(id(inst), ()))
                out.append(inst)
            bb2.instructions[:] = out


def _classify_mask(maskT):
    """Tile class per (ik sk-chunk, jq sq-block) of maskT [sk, sq]."""
    cls = {}
    sk = np.arange(S)[:, None]
    sq = np.arange(S)[None, :]
    causal = np.where(sq >= sk, 0.0, -1e9).astype(np.float32)
    for ik in range(NKC):
        for jq in range(NSB):
            t = maskT[ik * P:(ik + 1) * P, jq * SBLK:(jq + 1) * SBLK]
            if np.all(t <= -1e8):
                cls[(ik, jq)] = "SKIP"
            elif np.all(t == 0.0):
                cls[(ik, jq)] = "FREE"
            elif np.array_equal(
                    t, causal[ik * P:(ik + 1) * P, jq * SBLK:(jq + 1) * SBLK]):
                cls[(ik, jq)] = "DIAG"
            else:
                cls[(ik, jq)] = "MIX"
    return cls


def _build(cls_key, cls, no_cc=False):
    import concourse.bass as bass
    import concourse.mybir as mybir
    import concourse.tile as tile
    from concourse import masks

    _install_drain_patch(tile, mybir)
    f16, f32 = mybir.dt.float16, mybir.dt.float32
    nc = bass.Bass("TRN2", target_bir_lowering=False, debug=False,
                   num_devices=NC)

    di = lambda n, s: nc.dram_tensor(n, s, f16, kind="ExternalInput").ap()
    df = lambda n, s: nc.dram_tensor(n, s, f32, kind="ExternalInput").ap()

    xt = di("xt", [NSB, P, DCH, SBLK])      # packed xT fp16 per s-block
    xc = df("xc", [D, SHARD])               # xT fp32, this core's shard cols
    cost = df("cost", [P, S])               # rope cos  [hd, s]
    s2t = df("s2t", [P, S])                 # rope sign-folded sin [hd, s]
    wq = di("wq", [P, DCH, 2 * P])
    wk = di("wk", [P, DCH, P])
    wv = di("wv", [P, DCH, P])
    wo = di("wo", [DCH, P, DCH, P])         # [dtile][p][echunk][m]
    wg = di("wg", [2, P, DCH, SBLK])        # [ffhalf][p][dchunk][ff]
    wu = di("wu", [2, P, DCH, SBLK])
    wd = di("wd", [DCH, P, FF // P, P])     # [dtile][p][ffchunk][m]
    maskt = df("maskt", [S, S])
    dmask = df("dmask", [4, P, SBLK])
    outt = nc.dram_tensor("outt", [D, SHARD], f32, kind="ExternalOutput").ap()

    with tile.TileContext(nc) as tc:
        with (
            tc.tile_pool(name="pers", bufs=1) as pers,
            tc.tile_pool(name="dram", bufs=1, space="DRAM") as dram,
        ):
            ones = pers.tile([P, P], f16, name="ones")
            nc.vector.memset(ones[:], 1.0)
            ident = pers.tile([P, P], f16, name="ident")
            masks.make_identity(nc, ident[:])
            permT = pers.tile([P, P], f16, name="permT")  # rot-half permute
            nc.gpsimd.memset(permT[:], 0.0)
            for base in (-64, 64):
                nc.gpsimd.affine_select(
                    out=permT[:], in_=permT[:],
                    compare_op=mybir.AluOpType.not_equal,
                    fill=1.0, base=base, pattern=[[-1, P]],
                    channel_multiplier=1)

            shift = pers.tile([P, 1], f32, name="shift")
            nc.vector.memset(shift[:], EXP_SHIFT)
            dm_sb = pers.tile([P, 4, SBLK], f32, name="dm_sb")
            nc.sync.dma_start(dm_sb[:], dmask[:].rearrange("k p s -> p k s"))
            x1t = pers.tile([P, DCH, SHARD], f32, name="x1t")
            attn_rb = pers.tile([P, 2, NC, SHARD], f16, name="attn_rb")
            wgu_sb = pers.tile([P, 2, DCH, SBLK], f16, name="wgu_sb")

            a2a1_in = [dram.tile([NC, P, SHARD], f16, name=f"a2a1i{_h}") for _h in range(2)]
            a2a1_out = [dram.tile([NC, P, SHARD], f16, name=f"a2a1o{_h}") for _h in range(2)]
            ag2_in = dram.tile([D, SHARD], f16)
            ag2_out = dram.tile([NC * D, SHARD], f16, addr_space="Shared")
            a2a3_in = [dram.tile([NC, FFSH // 2, SHARD], f16, name=f"a2a3i{_h}") for _h in range(2)]
            a2a3_out = [dram.tile([NC, FFSH // 2, SHARD], f16, name=f"a2a3o{_h}") for _h in range(2)]

            # ======== A: norm1 stats + QKV + RoPE + v transpose ========
            _pAB_cm = tc.tile_pool(name="pAB", bufs=1)
            pAB = _pAB_cm.__enter__()
            qh = [pAB.tile([P, S], f16, name=f"qh{i}") for i in range(2)]
            kh = pAB.tile([P, S], f16, name="kh")
            vnat = pAB.tile([P, NKC, P], f16, name="vnat")
            attn_st = pAB.tile([P, 2, S], f16, name="attn_st")
            with (
                tc.tile_pool(name="pA", bufs=2) as pA,
                tc.tile_pool(name="pAx", bufs=2) as pAx,
                tc.tile_pool(name="pAw", bufs=1) as pAw,
                tc.tile_pool(name="psA", bufs=2, space="PSUM") as psA,
                tc.tile_pool(name="psAv", bufs=2, space="PSUM") as psAv,
            ):
                wq_sb0 = pAw.tile([P, DCH, 2 * P], f16, name="wq_sb")
                nc.sync.dma_start(wq_sb0[:], wq[:])
                wk_sb0 = pAw.tile([P, DCH, P], f16, name="wk_sb")
                nc.sync.dma_start(wk_sb0[:], wk[:])
                wv_sb0 = pAw.tile([P, DCH, P], f16, name="wv_sb")
                nc.sync.dma_start(wv_sb0[:], wv[:])
                wq_sb = [wq_sb0[:, _i] for _i in range(DCH)]
                wk_sb = [wk_sb0[:, _i] for _i in range(DCH)]
                wv_sb = [wv_sb0[:, _i] for _i in range(DCH)]

                for j in range(NSB):
                    sl = slice(j * SBLK, (j + 1) * SBLK)
                    xt_h0 = pAx.tile([P, DCH // 2, SBLK], f16, name="xt_h0")
                    nc.sync.dma_start(xt_h0[:], xt[j, :, :DCH // 2])
                    xt_h1 = pAx.tile([P, DCH // 2, SBLK], f16, name="xt_h1")
                    nc.sync.dma_start(xt_h1[:], xt[j, :, DCH // 2:])
                    xt_sb = [xt_h0[:, _i] for _i in range(DCH // 2)] + \
                            [xt_h1[:, _i] for _i in range(DCH // 2)]
                    var_ps = psAv.tile([P, SBLK], f32, name="var")
                    for i in range(DCH):
                        xsq = pA.tile([P, SBLK], f16, name="xsq")
                        nc.vector.tensor_mul(xsq[:], xt_sb[i][:],
                                             xt_sb[i][:])
                        nc.tensor.matmul(var_ps[:], ones[:], xsq[:],
                                         start=(i == 0), stop=(i == DCH - 1))
                    t1 = pA.tile([P, SBLK], f32, name="t1")
                    nc.vector.tensor_scalar(
                        t1[:], var_ps[:], 1.0 / D, EPS,
                        mybir.AluOpType.mult, mybir.AluOpType.add)
                    t2 = pA.tile([P, SBLK], f32, name="t2")
                    nc.vector.reciprocal(t2[:], t1[:])
                    rstd = pA.tile([P, SBLK], f32, name="rstd")
                    nc.scalar.sqrt(rstd[:], t2[:])
                    cosj = pA.tile([P, SBLK], f32, name="cosj")
                    nc.sync.dma_start(cosj[:], cost[:, sl])
                    s2j = pA.tile([P, SBLK], f32, name="s2j")
                    nc.sync.dma_start(s2j[:], s2t[:, sl])
                    cr = pA.tile([P, SBLK], f32, name="cr")
                    nc.vector.tensor_mul(cr[:], cosj[:], rstd[:])
                    sr = pA.tile([P, SBLK], f32, name="sr")
                    nc.vector.tensor_mul(sr[:], s2j[:], rstd[:])

                    for (wsb, col0, dst) in (
                        (wq_sb, 0, qh[0]), (wq_sb, P, qh[1]), (wk_sb, 0, kh)
                    ):
                        ps = psA.tile([P, SBLK], f32, name="mm")
                        for i in range(DCH):
                            nc.tensor.matmul(
                                ps[:], wsb[i][:, col0:col0 + P],
                                xt_sb[i][:],
                                start=(i == 0), stop=(i == DCH - 1))
                        z16 = pA.tile([P, SBLK], f16, name="z16")
                        nc.vector.tensor_copy(z16[:], ps[:])
                        rps = psA.tile([P, SBLK], f32, name="rot")
                        nc.tensor.matmul(rps[:], permT[:], z16[:],
                                         start=True, stop=True)
                        av = pA.tile([P, SBLK], f32, name="av")
                        nc.vector.tensor_mul(av[:], ps[:], cr[:])
                        bv = pA.tile([P, SBLK], f32, name="bv")
                        nc.vector.tensor_mul(bv[:], rps[:], sr[:])
                        nc.vector.tensor_add(dst[:, sl], av[:], bv[:])

                    ps = psA.tile([P, SBLK], f32, name="mm")
                    for i in range(DCH):
                        nc.tensor.matmul(ps[:], wv_sb[i][:], xt_sb[i][:],
                                         start=(i == 0), stop=(i == DCH - 1))
                    vs = pA.tile([P, SBLK], f16, name="vs")
                    nc.vector.tensor_mul(vs[:], ps[:], rstd[:])
                    for t in range(SBLK // P):
                        tps = psAv.tile([P, P], f16, name="vt")
                        nc.tensor.transpose(tps[:], vs[:, t * P:(t + 1) * P],
                                            ident[:])
                        nc.vector.tensor_copy(
                            vnat[:, j * (SBLK // P) + t, :], tps[:])

            _pDh_cm = tc.tile_pool(name="pDh", bufs=2)
            pDh = _pDh_cm.__enter__()
            _pWd_cm = tc.tile_pool(name="pWd", bufs=2)
            pWd = _pWd_cm.__enter__()
            # ======== B: attention in scoresT layout ========
            with (
                tc.tile_pool(name="pB", bufs=4) as pB,
                tc.tile_pool(name="psB", bufs=4, space="PSUM") as psB,
                tc.tile_pool(name="psBa", bufs=2, space="PSUM") as psBa,
            ):
                for h in range(2):
                    for jq in range(NSB):
                        slq = slice(jq * SBLK, (jq + 1) * SBLK)
                        live = [ik for ik in range(NKC)
                                if cls[(ik, jq)] != "SKIP"]
                        live.sort(key=lambda ik: 0 if cls[(ik, jq)] in
                                  ("DIAG", "MIX") else 1)
                        dn_ps = psBa.tile([P, SBLK], f32, name="dn")
                        at_ps = psBa.tile([P, SBLK], f32, name="at")
                        for n, ik in enumerate(live):
                            c = cls[(ik, jq)]
                            sc = psB.tile([P, SBLK], f32, name="sc")
                            nc.tensor.matmul(
                                sc[:], kh[:, ik * P:(ik + 1) * P],
                                qh[h][:, slq], start=True, stop=True)
                            pr = pB.tile([P, SBLK], f16, name="pr")
                            if c == "FREE":
                                nc.scalar.activation(
                                    pr[:], sc[:],
                                    mybir.ActivationFunctionType.Exp,
                                    bias=shift[:, :])
                            elif c == "DIAG":
                                r = ik - 4 * jq
                                assert 0 <= r < 4, (ik, jq)
                                cp = pB.tile([P, SBLK], f32, name="cp")
                                nc.vector.tensor_add(cp[:], sc[:],
                                                     dm_sb[:, r, :])
                                nc.scalar.activation(
                                    pr[:], cp[:],
                                    mybir.ActivationFunctionType.Exp,
                                    bias=shift[:, :])
                            else:  # MIX
                                mk = pB.tile([P, SBLK], f32, name="mk")
                                nc.sync.dma_start(
                                    mk[:], maskt[ik * P:(ik + 1) * P, slq])
                                cp = pB.tile([P, SBLK], f32, name="cp")
                                nc.vector.tensor_add(cp[:], sc[:], mk[:])
                                nc.scalar.activation(
                                    pr[:], cp[:],
                                    mybir.ActivationFunctionType.Exp,
                                    bias=shift[:, :])
                            nc.tensor.matmul(dn_ps[:], ones[:], pr[:],
                                             start=(n == 0),
                                             stop=(n == len(live) - 1))
                            nc.tensor.matmul(at_ps[:], vnat[:, ik, :], pr[:],
                                             start=(n == 0),
                                             stop=(n == len(live) - 1))
                        rc = pB.tile([P, SBLK], f32, name="rc")
                        nc.vector.reciprocal(rc[:], dn_ps[:])
                        nc.vector.tensor_mul(attn_st[:, h, slq], at_ps[:],
                                             rc[:])
                    nc.sync.dma_start(
                        a2a1_in[h][:].rearrange("c p s -> p c s"),
                        attn_st[:, h].rearrange("p (c s) -> p c s", c=NC))
                    if not no_cc:
                        nc.gpsimd.collective_compute(
                            "AllToAll", mybir.AluOpType.bypass,
                            replica_groups=[list(range(NC))],
                            ins=[a2a1_in[h][:].opt()],
                            outs=[a2a1_out[h][:].opt()])

            # ======== C: o_proj (seq-shard) + residual + norm2 ========
            with (
                tc.tile_pool(name="pC", bufs=3) as pC,
                tc.tile_pool(name="pCr", bufs=1) as pCr,
                tc.tile_pool(name="psC", bufs=2, space="PSUM") as psC,
                tc.tile_pool(name="psCv", bufs=1, space="PSUM") as psCv,
            ):
                for h in range(2):
                    nc.sync.dma_start(
                        attn_rb[:, h],
                        a2a1_out[h][:].rearrange("c p s -> p c s"))
                var2 = psCv.tile([P, SHARD], f32, name="var2")
                for i in range(DCH):
                    wo_sb = pC.tile([P, DCH, P], f16, name="wo_sb")
                    nc.sync.dma_start(wo_sb[:], wo[i])
                    ps = psC.tile([P, SHARD], f32, name="wops")
                    for h in range(2):
                        for cc in range(NC):
                            nc.tensor.matmul(
                                ps[:], wo_sb[:, 2 * cc + h, :],
                                attn_rb[:, h, cc, :],
                                start=(h == 0 and cc == 0),
                                stop=(h == 1 and cc == NC - 1))
                    xci = pC.tile([P, SHARD], f32, name="xci")
                    nc.sync.dma_start(
                        xci[:], xc[:].rearrange("(n p) s -> p n s", p=P)[:, i])
                    nc.vector.tensor_add(x1t[:, i, :], ps[:], xci[:])
                    sq2 = pC.tile([P, SHARD], f16, name="sq2")
                    nc.vector.tensor_mul(sq2[:], x1t[:, i, :], x1t[:, i, :])
                    nc.tensor.matmul(var2[:], ones[:], sq2[:],
                                     start=(i == 0), stop=(i == DCH - 1))
                u1 = pCr.tile([P, SHARD], f32, name="u1")
                nc.vector.tensor_scalar(
                    u1[:], var2[:], 1.0 / D, EPS,
                    mybir.AluOpType.mult, mybir.AluOpType.add)
                u2 = pCr.tile([P, SHARD], f32, name="u2")
                nc.vector.reciprocal(u2[:], u1[:])
                rstd2 = pCr.tile([P, SHARD], f32, name="rstd2")
                nc.scalar.sqrt(rstd2[:], u2[:])
                for i in range(DCH):
                    h2i = pC.tile([P, SHARD], f16, name="h2i")
                    nc.vector.tensor_mul(h2i[:], x1t[:, i, :], rstd2[:])
                    nc.sync.dma_start(
                        ag2_in[:].rearrange("(n p) s -> p n s", p=P)[:, i],
                        h2i[:])
            if not no_cc:
                nc.gpsimd.collective_compute(
                    "AllGather", mybir.AluOpType.bypass,
                    replica_groups=[list(range(NC))],
                    ins=[ag2_in[:].opt()], outs=[ag2_out[:].opt()])

            # ======== D: gate/up (ff-shard) + silu*up ========
            with (
                tc.tile_pool(name="pD", bufs=2) as pD,
                tc.tile_pool(name="psD", bufs=4, space="PSUM") as psD,
            ):
                for half in range(2):
                    wg_sb = wgu_sb[:, 0]
                    nc.sync.dma_start(wg_sb, wg[half])
                    wu_sb = wgu_sb[:, 1]
                    nc.sync.dma_start(wu_sb, wu[half])
                    for j in range(NSB):
                        h2rb = pDh.tile([P, DCH, 2, SHARD], f16, name="h2rb")
                        for cc in range(2):
                            nc.sync.dma_start(
                                h2rb[:, :, cc, :],
                                ag2_out[:].rearrange(
                                    "(c n p) s -> p n c s", c=NC, p=P
                                )[:, :, 2 * j + cc, :])
                        for ft in range(SBLK // P):
                            fsl = slice(ft * P, (ft + 1) * P)
                            psg = psD.tile([P, SBLK], f32, name="psg")
                            for i in range(DCH):
                                nc.tensor.matmul(
                                    psg[:], wg_sb[:, i, fsl], h2rb[:, i],
                                    start=(i == 0), stop=(i == DCH - 1))
                            psu = psD.tile([P, SBLK], f32, name="psu")
                            for i in range(DCH):
                                nc.tensor.matmul(
                                    psu[:], wu_sb[:, i, fsl], h2rb[:, i],
                                    start=(i == 0), stop=(i == DCH - 1))
                            slv = pD.tile([P, SBLK], f32, name="slv")
                            nc.scalar.activation(
                                slv[:], psg[:],
                                mybir.ActivationFunctionType.Silu)
                            gt = pD.tile([P, SBLK], f16, name="gt")
                            nc.vector.tensor_mul(gt[:], slv[:], psu[:])
                            nc.sync.dma_start(
                                a2a3_in[half][:].rearrange(
                                    "c (n p) s -> p n c s", p=P
                                )[:, ft, 2 * j:2 * j + 2, :],
                                gt[:].rearrange("p (c s) -> p c s", c=2))
                    if not no_cc:
                        nc.gpsimd.collective_compute(
                            "AllToAll", mybir.AluOpType.bypass,
                            replica_groups=[list(range(NC))],
                            ins=[a2a3_in[half][:].opt()],
                            outs=[a2a3_out[half][:].opt()])

            # ======== E: down_proj (seq-shard) + final residual ========
            with (
                tc.tile_pool(name="pE", bufs=2) as pE,
                tc.tile_pool(name="pEg", bufs=1) as pEg,
                tc.tile_pool(name="psE", bufs=2, space="PSUM") as psE,
            ):
                grb = [pEg.tile([P, FF // (2 * P), SHARD], f16,
                                name=f"grb{hf}") for hf in range(2)]
                for hf in range(2):
                    nc.sync.dma_start(
                        grb[hf][:],
                        a2a3_out[hf][:].rearrange("c (n p) s -> p (c n) s",
                                                  p=P))
                for i in range(DCH):
                    wd_sb = pWd.tile([P, FF // P, P], f16, name="wd_sb")
                    nc.sync.dma_start(wd_sb[:], wd[i])
                    ps = psE.tile([P, SHARD], f32, name="dps")
                    nmm = 0
                    for hf in range(2):
                        for cc in range(NC):
                            for n in range(4):
                                fg = cc * 8 + hf * 4 + n
                                nc.tensor.matmul(
                                    ps[:], wd_sb[:, fg, :],
                                    grb[hf][:, cc * 4 + n, :],
                                    start=(nmm == 0), stop=(nmm == 63))
                                nmm += 1
                    ot = pE.tile([P, SHARD], f32, name="ot")
                    nc.vector.tensor_add(ot[:], ps[:], x1t[:, i, :])
                    nc.sync.dma_start(
                        outt[:].rearrange("(n p) s -> p n s", p=P)[:, i], ot[:])
            _pWd_cm.__exit__(None, None, None)
            _pDh_cm.__exit__(None, None, None)
            _pAB_cm.__exit__(None, None, None)
    import concourse.mybir as _mybir
    _split_waits(nc, _mybir)
    return nc


def _host_prep(inputs):
    x = np.ascontiguousarray(inputs["hidden_states"][0])          # [S, D]
    mask = np.ascontiguousarray(inputs["attention_mask"][0, 0])   # [sq, sk]
    maskT = np.ascontiguousarray(mask.T)                          # [sk, sq]
    ln1, ln2 = inputs["ln1_w"], inputs["ln2_w"]
    Wq, Wk, Wv, Wo = inputs["Wq"], inputs["Wk"], inputs["Wv"], inputs["Wo"]
    Wg, Wu, Wd = inputs["Wg"], inputs["Wu"], inputs["Wd"]

    xT = np.ascontiguousarray(x.T)                                # [D, S]
    xT16 = xT.astype(np.float16)
    # packed xt: [j sblk][p][n dchunk][m] = xT[n*128+p, j*512+m]
    xtp = np.ascontiguousarray(
        xT16.reshape(DCH, P, NSB, SBLK).transpose(2, 1, 0, 3))

    inv_freq = 1.0 / (THETA ** (np.arange(0, HD, 2, dtype=np.float32) / HD))
    t = np.arange(S, dtype=np.float32)
    freqs = np.outer(t, inv_freq)
    emb = np.concatenate([freqs, freqs], -1)                      # [S, HD]
    cosT = np.ascontiguousarray(np.cos(emb).T.astype(np.float32))  # [HD, S]
    sinT = np.sin(emb).T.astype(np.float32)
    s2T = sinT.copy()
    s2T[:64] = -s2T[:64]
    s2T = np.ascontiguousarray(s2T)

    scale = 1.0 / np.sqrt(HD)
    Wq_f = (ln1[:, None] * Wq * scale).astype(np.float16)   # [D, H*HD]
    Wk_f = (ln1[:, None] * Wk).astype(np.float16)
    Wv_f = (ln1[:, None] * Wv).astype(np.float16)
    Wg_f = (ln2[:, None] * Wg).astype(np.float16)
    Wu_f = (ln2[:, None] * Wu).astype(np.float16)
    Wo16 = Wo.astype(np.float16)                            # [H*HD, D]
    Wd16 = Wd.astype(np.float16)                            # [FF, D]

    # packed wo: [i dtile][p][e chunk][m] = Wo[e*128+p, i*128+m]
    wop = np.ascontiguousarray(
        Wo16.reshape(DCH, P, DCH, P).transpose(2, 1, 0, 3))
    # packed wd: [i][p][f chunk][m] = Wd[f*128+p, i*128+m]
    wdp = np.ascontiguousarray(
        Wd16.reshape(FF // P, P, DCH, P).transpose(2, 1, 0, 3))

    cls = _classify_mask(maskT)
    dmask = np.zeros((4, P, SBLK), np.float32)
    for (ik, jq), c in cls.items():
        if c == "DIAG":
            r = ik - 4 * jq
            assert 0 <= r < 4, "DIAG tile off the ik==4*jq+r band"
            dmask[r] = maskT[ik * P:(ik + 1) * P, jq * SBLK:(jq + 1) * SBLK]
    in_maps = []
    for c in range(NC):
        qsl = slice(2 * P * c, 2 * P * (c + 1))
        kvsl = slice(P * (c // 2), P * (c // 2) + P)
        ffsl = slice(FFSH * c, FFSH * (c + 1))
        ssl = slice(SHARD * c, SHARD * (c + 1))
        wq_c = Wq_f[:, qsl]    # [D, 256]
        wk_c = Wk_f[:, kvsl]   # [D, 128]
        wv_c = Wv_f[:, kvsl]
        wg_c = Wg_f[:, ffsl]   # [D, 1024]
        wu_c = Wu_f[:, ffsl]
        in_maps.append({
            "xt": xtp,
            "xc": np.ascontiguousarray(xT[:, ssl]),
            "cost": cosT,
            "s2t": s2T,
            # [p][n dchunk][cols]
            "wq": np.ascontiguousarray(
                wq_c.reshape(DCH, P, 2 * P).transpose(1, 0, 2)),
            "wk": np.ascontiguousarray(
                wk_c.reshape(DCH, P, P).transpose(1, 0, 2)),
            "wv": np.ascontiguousarray(
                wv_c.reshape(DCH, P, P).transpose(1, 0, 2)),
            "wo": wop,
            # [half][p][n dchunk][ff 512] = Wg_f[n*128+p, half*512+m]
            "wg": np.ascontiguousarray(
                wg_c.reshape(DCH, P, 2, SBLK).transpose(2, 1, 0, 3)),
            "wu": np.ascontiguousarray(
                wu_c.reshape(DCH, P, 2, SBLK).transpose(2, 1, 0, 3)),
            "wd": wdp,
            "maskt": maskT,
            "dmask": dmask,
        })
    return in_maps, cls


def kernel(**inputs):
    from concourse import bass_utils

    in_maps, cls = _host_prep(inputs)
    cls_key = tuple(sorted(cls.items()))
    if cls_key not in _CACHE:
        _CACHE[cls_key] = _build(cls_key, cls)
    nc = _CACHE[cls_key]

    res = bass_utils.run_bass_kernel_spmd(
        nc, in_maps, core_ids=list(range(NC)))
    out = np.empty((S, D), dtype=np.float32)
    for c in range(NC):
        out[SHARD * c:SHARD * (c + 1), :] = res.results[c]["outt"].T
    return out[None]
